# revision 33
# baseline (speedup 1.0000x reference)
"""EnsembleUncertaintyDistance Trainium2 kernel.

out[q,p] = mean_m(o_mqp) * exp(-std_m(o_mqp)),
o_mqp = sum_h W2[m,h]*relu(hq[m,q,h]+hp[m,p,h]+b1[m,h]) + b2[m]

Strategy (8 cores, data-parallel over q: 256 rows each):
  - Host folds |W2| into W1/b1 (so the device reduce uses exact +-1 signs)
    and pre-transposes weights/activations for the d-contraction.
  - Device: hq/hp via PE matmuls; the big [m,q,p,h] outer-sum+relu via
    fused 2-op tensor_scalar on DVE (ACT co-produces a share) into bf16
    tiles [128=(2 models x 64 h), 256=p]; PE contracts each tile with a
    [128,2] sign matrix into per-(q, model) PSUM rows; model sums/sumsq
    via PE with 0/1 selection matrices; mean/std/exp tail on DVE+ACT.
"""
import threading
import time

import numpy as np
import ml_dtypes

EMB = 512
NM = 5
HID = 64
NQ = 2048
NP = 256
NCORES = 8
NQL = NQ // NCORES  # 256 local q rows
P = 128

F32 = None  # set lazily (mybir)
BF16 = None

_CACHE = {}
_LOCK = threading.Lock()


def _build_nc(repeat=1, act_every=3, producer_mode="ap", do_mm=True,
              do_stats=True):
    import concourse.mybir as mybir
    import concourse.tile as tile
    from concourse import bacc

    f32 = mybir.dt.float32
    f32r = mybir.dt.float32r
    bf16 = mybir.dt.bfloat16
    ADD = mybir.AluOpType.add
    MULT = mybir.AluOpType.mult
    SUB = mybir.AluOpType.subtract
    MAX = mybir.AluOpType.max
    RELU = mybir.ActivationFunctionType.Relu
    SQUARE = mybir.ActivationFunctionType.Square
    EXP = mybir.ActivationFunctionType.Exp
    LOG = mybir.ActivationFunctionType.Ln if hasattr(mybir.ActivationFunctionType, "Ln") else mybir.ActivationFunctionType.Log

    nc = bacc.Bacc("TRN2", target_bir_lowering=False, debug=False,
                   num_devices=NCORES)

    qT = nc.dram_tensor("qT", [EMB, NQL], f32, kind="ExternalInput").ap()
    pT = nc.dram_tensor("pT", [EMB, NP], f32, kind="ExternalInput").ap()
    w1qT = nc.dram_tensor("w1qT", [EMB, NM * HID], f32, kind="ExternalInput").ap()
    w1pT = nc.dram_tensor("w1pT", [EMB, NM * HID], f32, kind="ExternalInput").ap()
    b1c = nc.dram_tensor("b1c", [P, 3], f32, kind="ExternalInput").ap()
    wsel = nc.dram_tensor("wsel", [P, 3 * 16 * 32], bf16, kind="ExternalInput").ap()
    b2c = nc.dram_tensor("b2c", [P, 3], f32, kind="ExternalInput").ap()
    smat = nc.dram_tensor("smat", [P, 384], f32, kind="ExternalInput").ap()
    out = nc.dram_tensor("out", [NQL, NP], f32, kind="ExternalOutput").ap()

    with tile.TileContext(nc) as tc:
        with tc.tile_pool(name="const", bufs=1) as cpool, \
             tc.tile_pool(name="work", bufs=8) as tpool, \
             tc.tile_pool(name="spool", bufs=2) as spool, \
             tc.tile_pool(name="main_psum", bufs=2, space="PSUM") as mps:

            # ---- input DMA (outside repeat) ----
            qT_sb = cpool.tile([P, EMB // P, NQL], f32)
            pT_sb = cpool.tile([P, EMB // P, NP], f32)
            w1qT_sb = cpool.tile([P, EMB // P, NM * HID], f32)
            w1pT_sb = cpool.tile([P, EMB // P, NM * HID], f32)
            b1c_sb = cpool.tile([P, 3], f32)
            wsel_sb = cpool.tile([P, 3, 16, 32], bf16)
            b2c_sb = cpool.tile([P, 3], f32)
            smat_sb = cpool.tile([P, 384], f32)
            nc.sync.dma_start(qT_sb[:], qT.rearrange("(o p) q -> p o q", p=P))
            nc.sync.dma_start(pT_sb[:], pT.rearrange("(o p) q -> p o q", p=P))
            nc.sync.dma_start(w1qT_sb[:], w1qT.rearrange("(o p) q -> p o q", p=P))
            nc.sync.dma_start(w1pT_sb[:], w1pT.rearrange("(o p) q -> p o q", p=P))
            nc.sync.dma_start(b1c_sb[:], b1c)
            nc.sync.dma_start(wsel_sb[:], wsel.rearrange("p (a u c) -> p a u c", a=3, u=16))
            nc.sync.dma_start(b2c_sb[:], b2c)
            nc.sync.dma_start(smat_sb[:], smat)

            # warm the ACT table set (ln/exp + relu/square fillers) early
            scr = cpool.tile([1, 2], f32)
            nc.vector.memset(scr[:], 1.0)
            ln02 = cpool.tile([P, 1], f32)
            nc.vector.memset(ln02[:], float(np.log(0.2)))
            nc.scalar.activation(scr[:, 0:1], scr[:, 0:1], LOG)
            nc.scalar.activation(scr[:, 0:1], scr[:, 0:1], EXP)
            nc.scalar.activation(scr[:, 0:1], scr[:, 0:1], RELU)
            nc.scalar.activation(scr[:, 0:1], scr[:, 0:1], SQUARE)

            for rep in range(repeat):
                # ---- precompute hq/hp on PE ----
                hq_ps01 = mps.tile([P, 2, NQL], f32, tag="bank_01")
                hp_ps01 = mps.tile([P, 2, NP], f32, tag="bank_23")
                hq_ps23 = mps.tile([P, 2, NQL], f32, tag="bank_44")
                hp_ps23 = mps.tile([P, 2, NP], f32, tag="bank_s")

                def hmm(ps_out, w_sb, x_sb, m):
                    for k in range(EMB // P):
                        nc.tensor.matmul(
                            ps_out,
                            w_sb[:, k, m * HID:(m + 1) * HID],
                            x_sb[:, k, :],
                            start=(k == 0), stop=(k == EMB // P - 1))

                # ---- producer operands in SBUF ----
                hq01 = cpool.tile([P, NQL], f32, tag="hq01")
                hq23 = cpool.tile([P, NQL], f32, tag="hq23")
                hq4p = cpool.tile([P, NQL // 2], f32, tag="hq4p")
                hpb01 = cpool.tile([P, NP], bf16, tag="hpb01")
                hpb23 = cpool.tile([P, NP], bf16, tag="hpb23")
                hpb4d = cpool.tile([P, NP], bf16, tag="hpb4d")

                # pair tiles: [0:64]=model a, [64:128]=model b
                hmm(hq_ps01[0:HID, 0, :], w1qT_sb, qT_sb, 0)
                hmm(hq_ps01[HID:P, 0, :], w1qT_sb, qT_sb, 1)
                hmm(hq_ps23[0:HID, 0, :], w1qT_sb, qT_sb, 2)
                hmm(hq_ps23[HID:P, 0, :], w1qT_sb, qT_sb, 3)
                # model 4 duplicated into both halves (for q-pair packing)
                hmm(hq_ps01[0:HID, 1, :], w1qT_sb, qT_sb, 4)
                hmm(hq_ps01[HID:P, 1, :], w1qT_sb, qT_sb, 4)

                hmm(hp_ps01[0:HID, 0, :], w1pT_sb, pT_sb, 0)
                hmm(hp_ps01[HID:P, 0, :], w1pT_sb, pT_sb, 1)
                hmm(hp_ps23[0:HID, 0, :], w1pT_sb, pT_sb, 2)
                hmm(hp_ps23[HID:P, 0, :], w1pT_sb, pT_sb, 3)
                hmm(hp_ps01[0:HID, 1, :], w1pT_sb, pT_sb, 4)
                hmm(hp_ps01[HID:P, 1, :], w1pT_sb, pT_sb, 4)

                nc.vector.tensor_copy(out=hq01[:], in_=hq_ps01[:, 0, :])
                nc.vector.tensor_copy(out=hq23[:], in_=hq_ps23[:, 0, :])
                # q-pairs for model 4: top half <- even q (from dup top),
                # bottom half <- odd q (from dup bottom); lanes stay put.
                hq4_pairs = hq_ps01[:, 1, :].rearrange("p (q two) -> p two q", two=2)
                nc.vector.tensor_copy(out=hq4p[0:HID, :], in_=hq4_pairs[0:HID, 0, :])
                nc.vector.tensor_copy(out=hq4p[HID:P, :], in_=hq4_pairs[HID:P, 1, :])

                nc.vector.tensor_scalar(hpb01[:], hp_ps01[:, 0, :], b1c_sb[:, 0:1],
                                        None, ADD)
                nc.vector.tensor_scalar(hpb23[:], hp_ps23[:, 0, :], b1c_sb[:, 1:2],
                                        None, ADD)
                nc.vector.tensor_scalar(hpb4d[:], hp_ps01[:, 1, :], b1c_sb[:, 2:3],
                                        None, ADD)

                # ---- main loop ----
                pairs = [(hq01, hpb01, 0), (hq23, hpb23, 2)]
                tile_idx = 0
                for sb in range(NQL // P):
                    # one accumulation target per bank at a time:
                    # groups within a bank are strictly sequential.
                    bank_01 = mps.tile([P, 2, NP], f32, tag="bank_01")  # out01A | out01B
                    bank_23 = mps.tile([P, 2, NP], f32, tag="bank_23")  # out23A | out23B
                    bank_44 = mps.tile([P, 2, NP], f32, tag="bank_44")  # out44 | sum
                    bank_s = mps.tile([P, 2, NP], f32, tag="bank_s")    # sumsq | spare
                    outs01 = [bank_01[:, 0, :], bank_01[:, 1, :]]
                    outs23 = [bank_23[:, 0, :], bank_23[:, 1, :]]
                    out44 = bank_44[:, 0, :]

                    static_t = []
                    if producer_mode == "none":
                        for si in range(4):
                            st = tpool.tile([P, NP], bf16, tag=f"st{si}",
                                            name=f"st{si}_{sb}_{rep}")
                            nc.vector.tensor_scalar(st[:], hpb01[:], 0.5,
                                                    0.0, ADD, MAX)
                            static_t.append(st)

                    def produce(hq_t, hpb_t, col):
                        nonlocal tile_idx
                        i = tile_idx
                        tile_idx += 1
                        if producer_mode == "none":
                            return static_t[i % 4]
                        t = tpool.tile([P, NP], bf16, tag="t")
                        if producer_mode == "imm":
                            nc.vector.tensor_scalar(t[:], hpb_t[:], 0.5,
                                                    0.0, ADD, MAX)
                        elif i % act_every == act_every - 1:
                            nc.scalar.activation(t[:], hpb_t[:], RELU,
                                                 bias=hq_t[:, col:col + 1])
                        else:
                            nc.vector.tensor_scalar(
                                t[:], hpb_t[:], hq_t[:, col:col + 1], 0.0, ADD, MAX)
                        return t

                    # interleave the three banks so consecutive PE matmuls
                    # never target the same PSUM bank (keeps fill/drain
                    # overlap; per-pass order measured 1.5x slower).
                    for j in range(P):
                        q = sb * P + j
                        half, jj = divmod(j, HID)
                        g, u = divmod(jj, 16)
                        for (hq_t, hpb_t, pi) in pairs:
                            t = produce(hq_t, hpb_t, q)
                            dst = (outs01 if pi == 0 else outs23)[half]
                            if do_mm:
                                nc.tensor.matmul(dst[32 * g:32 * (g + 1), :],
                                                 wsel_sb[:, pi // 2, u, :], t[:],
                                                 start=(u == 0), stop=(u == 15),
                                                 tile_position=(0, 32 * g),
                                                 skip_group_check=True)
                        if j % 2 == 0:
                            w = j // 2
                            g4, v = divmod(w, 16)
                            qp = sb * (P // 2) + w
                            t = produce(hq4p, hpb4d, qp)
                            if do_mm:
                                nc.tensor.matmul(out44[32 * g4:32 * (g4 + 1), :],
                                                 wsel_sb[:, 2, v, :], t[:],
                                                 start=(v == 0), stop=(v == 15),
                                                 tile_position=(0, 32 * g4),
                                                 skip_group_check=True)
                    if not do_mm:
                        # keep banks written so stats still have data
                        nc.vector.memset(bank_01[:], 0.1)
                        nc.vector.memset(bank_23[:], 0.1)
                        nc.vector.memset(bank_44[:, 0, :], 0.1)
                    if not do_stats:
                        res0 = spool.tile([P, NP], f32, tag="res")
                        nc.vector.tensor_scalar(res0[:], bank_01[:, 0, :], 0.0,
                                                None, ADD)
                        nc.sync.dma_start(out[sb * P:(sb + 1) * P, :], res0[:])
                        continue

                    # ---- stats for this superblock ----
                    # drain + b2  (DVE), squares (ACT)
                    o01 = spool.tile([P, 2, NP], f32, tag="o01")
                    o23 = spool.tile([P, 2, NP], f32, tag="o23")
                    o44 = spool.tile([P, NP], f32, tag="o44")
                    sq01 = spool.tile([P, 2, NP], f32, tag="sq01")
                    sq23 = spool.tile([P, 2, NP], f32, tag="sq23")
                    sq44 = spool.tile([P, NP], f32, tag="sq44")
                    # drains+b2 on DVE; squares as DVE TT on the drained SBUF
                    # tiles (keeps ACT on a single ln/exp/relu table set).
                    for h in range(2):
                        nc.vector.tensor_scalar(o01[:, h, :], outs01[h],
                                                b2c_sb[:, 0:1], None, ADD)
                        nc.vector.tensor_scalar(o23[:, h, :], outs23[h],
                                                b2c_sb[:, 1:2], None, ADD)
                    nc.vector.tensor_scalar(o44[:], out44, b2c_sb[:, 2:3], None, ADD)
                    nc.vector.tensor_tensor(sq01[:], o01[:], o01[:], MULT)
                    nc.vector.tensor_tensor(sq23[:], o23[:], o23[:], MULT)
                    nc.vector.tensor_tensor(sq44[:], o44[:], o44[:], MULT)

                    # sums over models via PE, one sequential 5-MM group per
                    # bank (partition i of the result = q offset i in the sb).
                    psum5 = bank_44[:, 1, :]
                    psq5 = bank_s[:, 0, :]
                    P_A = smat_sb[:, 0:P]
                    P_B = smat_sb[:, P:2 * P]
                    I128 = smat_sb[:, 2 * P:3 * P]
                    for (dst_ps, srcs, src44) in ((psum5, (o01, o23), o44),
                                                  (psq5, (sq01, sq23), sq44)):
                        nc.tensor.matmul(dst_ps, P_A, srcs[0][:, 0, :],
                                         start=True, stop=False)
                        nc.tensor.matmul(dst_ps, P_A, srcs[1][:, 0, :],
                                         start=False, stop=False)
                        nc.tensor.matmul(dst_ps, P_B, srcs[0][:, 1, :],
                                         start=False, stop=False)
                        nc.tensor.matmul(dst_ps, P_B, srcs[1][:, 1, :],
                                         start=False, stop=False)
                        nc.tensor.matmul(dst_ps, I128, src44,
                                         start=False, stop=True)

                    # mean/std/exp tail
                    s2 = spool.tile([P, NP], f32, tag="s2")
                    d = spool.tile([P, NP], f32, tag="d")
                    l = spool.tile([P, NP], f32, tag="l")
                    e2 = spool.tile([P, NP], f32, tag="e2")
                    res = spool.tile([P, NP], f32, tag="res")
                    # s2 = 0.2*sum^2 = (sqrt(0.2)*sum)^2 without touching ACT
                    su2 = spool.tile([P, NP], f32, tag="su2")
                    nc.vector.tensor_scalar(su2[:], psum5, float(np.sqrt(0.2)),
                                            None, MULT)
                    nc.vector.tensor_tensor(s2[:], su2[:], su2[:], MULT)
                    nc.vector.tensor_tensor(d[:], psq5, s2[:], SUB)
                    # d = 4*var ; ln(var) = ln(0.25*d)
                    nc.scalar.activation(l[:], d[:], LOG, scale=0.25)
                    nc.scalar.activation(s2[:], l[:], EXP, scale=0.5)  # std
                    nc.scalar.activation(e2[:], s2[:], EXP, scale=-1.0,
                                         bias=ln02[:, 0:1])
                    nc.vector.tensor_tensor(res[:], e2[:], psum5, MULT)
                    nc.sync.dma_start(out[sb * P:(sb + 1) * P, :], res[:])

    nc.compile()
    return nc


class _Runner:
    def __init__(self, nc, n_cores=NCORES):
        import jax
        from jax.sharding import Mesh, PartitionSpec
        from jax.experimental.shard_map import shard_map
        import concourse.mybir as mybir
        from concourse import bass2jax

        bass2jax.install_neuronx_cc_hook()
        self.jax = jax
        self.n_cores = n_cores
        self.in_names, self.out_names, out_avals, self.zero_outs = [], [], [], []
        pname = nc.partition_id_tensor.name if nc.partition_id_tensor else None
        for alloc in nc.m.functions[0].allocations:
            if not isinstance(alloc, mybir.MemoryLocationSet):
                continue
            name = alloc.memorylocations[0].name
            if alloc.kind == "ExternalInput":
                if name != pname:
                    self.in_names.append(name)
            elif alloc.kind == "ExternalOutput":
                self.out_names.append(name)
                shape = tuple(alloc.tensor_shape)
                dtype = mybir.dt.np(alloc.dtype)
                out_avals.append(jax.core.ShapedArray(shape, dtype))
                self.zero_outs.append(np.zeros(shape, dtype))
        n_params = len(self.in_names)
        n_outs = len(out_avals)
        all_names = tuple(self.in_names + self.out_names + ([pname] if pname else []))
        out_names = tuple(self.out_names)

        def _body(*args):
            operands = list(args)
            if pname is not None:
                operands.append(bass2jax.partition_id_tensor())
            return tuple(bass2jax._bass_exec_p.bind(
                *operands, out_avals=tuple(out_avals), in_names=all_names,
                out_names=out_names, lowering_input_output_aliases=(),
                sim_require_finite=True, sim_require_nnan=True, nc=nc))

        devices = jax.devices()[:n_cores]
        mesh = Mesh(np.asarray(devices), ("core",))
        self.fn = jax.jit(
            shard_map(_body, mesh=mesh,
                      in_specs=(PartitionSpec("core"),) * (n_params + n_outs),
                      out_specs=(PartitionSpec("core"),) * n_outs,
                      check_rep=False),
            keep_unused=True)

    def concat_inputs(self, in_maps):
        cat = [np.concatenate([np.asarray(m[name]) for m in in_maps], axis=0)
               for name in self.in_names]
        cat += [np.zeros((self.n_cores * z.shape[0], *z.shape[1:]), z.dtype)
                for z in self.zero_outs]
        return cat

    def run(self, in_maps):
        outs = self.fn(*self.concat_inputs(in_maps))
        self.jax.block_until_ready(outs)
        res = []
        for c in range(self.n_cores):
            d = {}
            for i, name in enumerate(self.out_names):
                full = np.asarray(outs[i])
                per = full.shape[0] // self.n_cores
                d[name] = full[c * per:(c + 1) * per]
            res.append(d)
        return res

    def time_it(self, in_maps, iters=20):
        args = [self.jax.device_put(x) for x in self.concat_inputs(in_maps)]
        outs = self.fn(*args)
        self.jax.block_until_ready(outs)
        times = []
        for _ in range(iters):
            t0 = time.perf_counter()
            outs = self.fn(*args)
            self.jax.block_until_ready(outs)
            times.append((time.perf_counter() - t0) * 1e9)
        times.sort()
        return times


def _host_prep(query_features, prototypes, W1, b1, W2, b2):
    """Host-side layout prep; returns per-core in_maps."""
    q = np.asarray(query_features, np.float32)
    p = np.asarray(prototypes, np.float32)
    W1 = np.asarray(W1, np.float32)
    b1 = np.asarray(b1, np.float32)
    W2 = np.asarray(W2, np.float32)
    b2 = np.asarray(b2, np.float32)

    absW2 = np.abs(W2)                       # [M, H]
    sgnW2 = np.where(W2 >= 0, 1.0, -1.0).astype(np.float32)
    W1q = W1[:, :, :EMB] * absW2[:, :, None]  # [M, H, E]
    W1p = W1[:, :, EMB:] * absW2[:, :, None]
    b1s = b1 * absW2                         # [M, H]

    qT = np.ascontiguousarray(q.T)           # [E, NQ]
    pT = np.ascontiguousarray(p.T)           # [E, NP]
    # [E, M*H]: model-major along free dim
    w1qT = np.ascontiguousarray(W1q.transpose(2, 0, 1).reshape(EMB, NM * HID))
    w1pT = np.ascontiguousarray(W1p.transpose(2, 0, 1).reshape(EMB, NM * HID))

    b1c = np.zeros((P, 3), np.float32)
    b1c[:HID, 0], b1c[HID:, 0] = b1s[0], b1s[1]
    b1c[:HID, 1], b1c[HID:, 1] = b1s[2], b1s[3]
    b1c[:HID, 2], b1c[HID:, 2] = b1s[4], b1s[4]

    # wsel[:, pi, u, :]: [128, 32] sign matrix with only cols (2u, 2u+1)
    # nonzero -> 16 q's accumulate into one 32-row PSUM group.
    wsel = np.zeros((P, 3, 16, 32), np.float32)
    model_pairs = ((0, 1), (2, 3), (4, 4))
    for pi, (ma, mb) in enumerate(model_pairs):
        for u in range(16):
            wsel[:HID, pi, u, 2 * u] = sgnW2[ma]
            wsel[HID:, pi, u, 2 * u + 1] = sgnW2[mb]
    wsel = wsel.reshape(P, 3 * 16 * 32).astype(ml_dtypes.bfloat16)

    b2c = np.zeros((P, 3), np.float32)
    b2c[0::2, 0], b2c[1::2, 0] = b2[0], b2[1]
    b2c[0::2, 1], b2c[1::2, 1] = b2[2], b2[3]
    b2c[:, 2] = b2[4]

    # smat: [P_A | P_B | I128], each [128, 128].
    # pair tiles: partition 32*(i//16) + 2*(i%16) + m holds q-offset i of the
    # half; P_A maps an A-half tile to sum partitions 0:64, P_B to 64:128.
    smat = np.zeros((P, 384), np.float32)
    for i in range(64):
        r = 32 * (i // 16) + 2 * (i % 16)
        smat[r, i] = 1.0
        smat[r + 1, i] = 1.0
        smat[r, P + 64 + i] = 1.0
        smat[r + 1, P + 64 + i] = 1.0
    for k in range(P):
        smat[k, 2 * P + k] = 1.0     # I128 for the model-4 tile (partition=q)
    shared = dict(pT=pT, w1qT=w1qT, w1pT=w1pT, b1c=b1c, wsel=wsel,
                  b2c=b2c, smat=smat)
    in_maps = []
    for c in range(NCORES):
        m = dict(shared)
        m["qT"] = np.ascontiguousarray(qT[:, c * NQL:(c + 1) * NQL])
        in_maps.append(m)
    return in_maps


def _get_runner(repeat=1, act_every=3):
    key = (repeat, act_every)
    with _LOCK:
        if key not in _CACHE:
            nc = _build_nc(repeat=repeat, act_every=act_every)
            _CACHE[key] = _Runner(nc)
        return _CACHE[key]


def kernel(query_features, prototypes, W1, b1, W2, b2):
    in_maps = _host_prep(query_features, prototypes, W1, b1, W2, b2)
    runner = _get_runner()
    res = runner.run(in_maps)
    return np.concatenate([res[c]["out"] for c in range(NCORES)], axis=0)
